# revision 15
# baseline (speedup 1.0000x reference)
"""GAT + actor-critic fused Trainium2 kernel, 8-core edge-parallel.

Sharding: edges partitioned by dst node range (N/8 nodes per core) so the
attention segment-softmax is core-local. Node phase (h = x@W) replicated.
Segment sums run on the TensorEngine with host-precomputed 0/1 dst-indicator
matrices (race-free). One AllGather shares per-core p_src projections (plus
critic partial sums) before the per-edge actor MLP.

Edges are gathered with SWDGE dma_gather (int16 indices): each 128-node dst
window keeps its edges in two fixed-size runs split by src < 32768 so the
gather table fits the int16 index range; per-window er values are fetched
with a [128,1]-offset indirect DMA and staged into a core-local table.
"""
import sys
for _p in ('/opt/trn_rl_repo', '/opt/pypackages'):
    if _p not in sys.path:
        sys.path.insert(0, _p)

import numpy as np
import ml_dtypes

import concourse.bass as bass
import concourse.mybir as mybir
import concourse.tile as tile
import concourse.bacc as bacc
from concourse import library_config

F32 = mybir.dt.float32
BF16 = mybir.dt.bfloat16
I32 = mybir.dt.int32
I16 = mybir.dt.int16
AF = mybir.ActivationFunctionType
ALU = mybir.AluOpType

NEG_SLOPE = 0.2
N_CORES = 8
PAIR = 2          # windows fused per phase-A iteration
TB = 4096         # phase-B edges per tile
THR = 32768


def _wrap16(idx, T):
    a = np.zeros((16, T // 16), np.int16)
    a[np.arange(len(idx)) % 16, np.arange(len(idx)) // 16] = idx.astype(np.int16)
    return np.tile(a, (8, 1))


def _prep(node_features, src, dst, fc_w, attn_l, attn_r, gat_bias,
          a_w1, a_b1, a_w2, a_b2, c_w1, c_b1, c_w2, c_b2):
    N, IN = node_features.shape
    E = src.shape[0]
    H, Dh = attn_l.shape
    HD = H * Dh
    NL = N // N_CORES
    WN = 128
    NW = (NL + WN - 1) // WN
    CROWS = NW * WN + 128

    # --- weights ----------------------------------------------------------
    al_mat = np.zeros((HD, H), np.float32)
    ar_mat = np.zeros((HD, H), np.float32)
    for h in range(H):
        al_mat[h * Dh:(h + 1) * Dh, h] = attn_l[h]
        ar_mat[h * Dh:(h + 1) * Dh, h] = attn_r[h]
    w_big = np.concatenate([fc_w, fc_w @ al_mat, fc_w @ ar_mat], axis=1)
    NPAD = ((N + 127) // 128) * 128
    xT = np.zeros((IN, NPAD), np.float32)
    xT[:, :N] = node_features.T
    wp = np.concatenate([a_w1[:HD], a_w1[HD:]], axis=1)
    wp_rs = wp.reshape(2, 128, 128).transpose(1, 0, 2).reshape(128, 256)
    cw1_rs = (c_w1 / N).reshape(4, 64, 64).transpose(1, 0, 2).reshape(64, 256)

    consts = dict(
        xT=xT.astype(ml_dtypes.bfloat16),
        w_big=w_big.astype(ml_dtypes.bfloat16),
        wp=wp_rs.astype(ml_dtypes.bfloat16),
        w2=a_w2.astype(ml_dtypes.bfloat16),
        b1=a_b1.reshape(-1, 1).astype(np.float32),
        b2=a_b2.reshape(-1, 1).astype(np.float32),
        bias_rep=np.tile(gat_bias.reshape(1, HD), (128, 1)).astype(np.float32),
        cw1=cw1_rs.astype(ml_dtypes.bfloat16),
        cb1=c_b1.reshape(-1, 1).astype(np.float32),
        cw2=c_w2.reshape(64, 1).astype(ml_dtypes.bfloat16),
        cb2=c_b2.reshape(1, 1).astype(np.float32),
        sel=np.kron(np.ones((N_CORES, 1), np.float32),
                    np.eye(4, dtype=np.float32)).astype(ml_dtypes.bfloat16),
        ident=np.eye(128, dtype=np.float32).astype(ml_dtypes.bfloat16),
    )

    # --- phase A: per (core, window) edge runs split by src < THR ---------
    core_of = dst // NL
    per_core = [np.nonzero(core_of == k)[0] for k in range(N_CORES)]

    winlists = [[None] * NW for _ in range(N_CORES)]
    WE_L = WE_H = 128
    for k in range(N_CORES):
        eids = per_core[k]
        w_of = (dst[eids] - k * NL) // WN
        for w in range(NW):
            l = eids[w_of == w]
            lo = l[src[l] < THR]
            hi = l[src[l] >= THR]
            winlists[k][w] = (lo, hi)
            WE_L = max(WE_L, len(lo))
            WE_H = max(WE_H, len(hi))
    WE_L = ((WE_L + 127) // 128) * 128
    WE_H = ((WE_H + 127) // 128) * 128
    NCHL, NCHH = WE_L // 128, WE_H // 128
    NCH = NCHL + NCHH
    NIT = (NW + PAIR - 1) // PAIR

    idxL = np.zeros((N_CORES, NIT, 128, PAIR * WE_L // 16), np.int16)
    idxH = np.zeros((N_CORES, NIT, 128, PAIR * WE_H // 16), np.int16)
    idxE = np.zeros((N_CORES, NIT, 128, PAIR * NCH * 8), np.int16)
    idxW = np.zeros((N_CORES, NW * 128, 1), np.int32)
    Bc = np.zeros((N_CORES, NW, 128, NCH * 128), ml_dtypes.bfloat16)
    for k in range(N_CORES):
        for w in range(NW):
            base = min(k * NL + w * WN, NPAD - WN)
            idxW[k, w * 128:(w + 1) * 128, 0] = base + np.arange(WN)
        for it in range(NIT):
            wins = [w for w in (it * PAIR, it * PAIR + 1) if w < NW]
            nwin = len(wins)
            sl = np.zeros(nwin * WE_L, np.int64)
            sh = np.zeros(nwin * WE_H, np.int64)
            de = np.zeros(nwin * NCH * 128, np.int64)
            for wi, w in enumerate(wins):
                lo, hi = winlists[k][w]
                sl[wi * WE_L:wi * WE_L + len(lo)] = src[lo]
                sh[wi * WE_H:wi * WE_H + len(hi)] = src[hi] - THR
                # global slot order: [w0 L | w1 L | w0 H | w1 H]
                de[wi * WE_L:wi * WE_L + len(lo)] = dst[lo] - k * NL
                hb = nwin * WE_L + wi * WE_H
                de[hb:hb + len(hi)] = dst[hi] - k * NL
                for grp, js in ((lo, 0), (hi, NCHL)):
                    pos = np.arange(len(grp))
                    p, j = pos % 128, pos // 128
                    Bc[k, w, p, (js + j) * 128 +
                       (dst[grp] - k * NL - w * WN)] = 1.0
            idxL[k, it, :, :nwin * WE_L // 16] = _wrap16(sl, nwin * WE_L)
            idxH[k, it, :, :nwin * WE_H // 16] = _wrap16(sh, nwin * WE_H)
            idxE[k, it, :, :nwin * NCH * 8] = _wrap16(de, nwin * NCH * 128)

    # --- phase B ----------------------------------------------------------
    msrc = (src // NL) * CROWS + (src % NL)
    low_sets, high_sets = [], []
    maxL = maxH = 1
    for k in range(N_CORES):
        m = msrc[per_core[k]]
        lo = per_core[k][m < THR]
        hi = per_core[k][m >= THR]
        low_sets.append(lo); high_sets.append(hi)
        maxL = max(maxL, len(lo)); maxH = max(maxH, len(hi))
    NBL = (maxL + TB - 1) // TB
    NBH = (maxH + TB - 1) // TB
    NB = NBL + NBH

    idx_ps = np.zeros((N_CORES, NB, 128, TB // 16), np.int16)
    idx_pd = np.zeros((N_CORES, NB, 128, TB // 16), np.int16)
    slotB = -np.ones((N_CORES, NB * TB), np.int64)
    for k in range(N_CORES):
        for part, bt, nt in ((low_sets[k], 0, NBL), (high_sets[k], NBL, NBH)):
            m = msrc[part] - (THR if bt else 0)
            d_loc = dst[part] - k * NL
            mp = np.zeros(nt * TB, np.int64); mp[:len(part)] = m
            dp = np.zeros(nt * TB, np.int64); dp[:len(part)] = d_loc
            for t in range(nt):
                s_ = slice(t * TB, (t + 1) * TB)
                idx_ps[k, bt + t] = _wrap16(mp[s_], TB)
                idx_pd[k, bt + t] = _wrap16(dp[s_], TB)
            slotB[k, bt * TB:bt * TB + len(part)] = part

    dims = dict(N=N, IN=IN, E=E, HD=HD, NL=NL, WN=WN, NW=NW, NIT=NIT,
                WE_L=WE_L, WE_H=WE_H, NCHL=NCHL, NCHH=NCHH, NCH=NCH,
                CROWS=CROWS, NBL=NBL, NBH=NBH, NB=NB, NPAD=NPAD)
    per_core_inputs = []
    for k in range(N_CORES):
        d = dict(consts)
        d.update(
            idxL=idxL[k].reshape(NIT * 128, PAIR * WE_L // 16),
            idxH=idxH[k].reshape(NIT * 128, PAIR * WE_H // 16),
            idxE=idxE[k].reshape(NIT * 128, PAIR * NCH * 8),
            idxW=idxW[k],
            Bc=Bc[k].reshape(NW * 128, NCH * 128),
            idx_ps=idx_ps[k].reshape(NB * 128, TB // 16),
            idx_pd=idx_pd[k].reshape(NB * 128, TB // 16),
        )
        per_core_inputs.append(d)
    return dims, per_core_inputs, slotB


# ----------------------------------------------------------------------------
# device program
# ----------------------------------------------------------------------------

def build(dims):
    IN, HD = dims['IN'], dims['HD']
    NW, WN = dims['NW'], dims['WN']
    NIT, WE_L, WE_H = dims['NIT'], dims['WE_L'], dims['WE_H']
    NCHL, NCHH, NCH = dims['NCHL'], dims['NCHH'], dims['NCH']
    CROWS, NB, NBL = dims['CROWS'], dims['NB'], dims['NBL']
    NPAD = dims['NPAD']
    HE = HD + 4          # h | el
    HROW = 384           # h_tab row elems (768B)
    HER = HD + 8

    nc = bacc.Bacc(None, target_bir_lowering=False)
    Pi = lambda n, s, d: nc.declare_dram_parameter(n, s, d, isOutput=False)
    xT = Pi("xT", [IN, NPAD], BF16)
    w_big = Pi("w_big", [IN, HER], BF16)
    wp = Pi("wp", [128, 256], BF16)
    w2 = Pi("w2", [64, 16], BF16)
    b1 = Pi("b1", [64, 1], F32)
    b2 = Pi("b2", [16, 1], F32)
    bias_rep = Pi("bias_rep", [128, HD], F32)
    cw1 = Pi("cw1", [64, 256], BF16)
    cb1 = Pi("cb1", [64, 1], F32)
    cw2 = Pi("cw2", [64, 1], BF16)
    cb2 = Pi("cb2", [1, 1], F32)
    selp = Pi("sel", [32, 4], BF16)
    ident_p = Pi("ident", [128, 128], BF16)
    idxL = Pi("idxL", [NIT * 128, PAIR * WE_L // 16], I16)
    idxH = Pi("idxH", [NIT * 128, PAIR * WE_H // 16], I16)
    idxE = Pi("idxE", [NIT * 128, PAIR * NCH * 8], I16)
    idxW = Pi("idxW", [NW * 128, 1], I32)
    Bc = Pi("Bc", [NW * 128, NCH * 128], BF16)
    idx_ps = Pi("idx_ps", [NB * 128, TB // 16], I16)
    idx_pd = Pi("idx_pd", [NB * 128, TB // 16], I16)

    out_logits = nc.declare_dram_parameter("out_logits", [16, NB * TB], F32,
                                           isOutput=True)
    out_sval = nc.declare_dram_parameter("out_sval", [1, 1], F32, isOutput=True)

    h_tab = nc.dram_tensor("h_tab", [NPAD, HROW], BF16)
    er_tab = nc.dram_tensor("er_tab", [NPAD, 4], F32)
    er_loc = nc.dram_tensor("er_loc", [NW * WN, 64], F32)
    contrib = nc.dram_tensor("contrib", [CROWS, 128], BF16)
    ps_tab = nc.dram_tensor("ps_tab", [N_CORES * CROWS, 128], BF16,
                            addr_space="Shared")
    pd_tab = nc.dram_tensor("pd_tab", [NW * WN, 128], BF16)

    idxL_v = idxL.ap().rearrange("(i p) c -> p i c", p=128)
    idxH_v = idxH.ap().rearrange("(i p) c -> p i c", p=128)
    idxE_v = idxE.ap().rearrange("(i p) c -> p i c", p=128)
    Bc_v = Bc.ap().rearrange("(w p) c -> p w c", p=128)

    with tile.TileContext(nc) as tc:
        with (
            tc.tile_pool(name="persist", bufs=1) as pers,
            tc.tile_pool(name="io", bufs=3) as io,
            tc.tile_pool(name="psum", bufs=2, space="PSUM") as psp,
        ):
            wbig_sb = pers.tile([IN, HER], BF16, tag="wbig_sb")
            wp_sb = pers.tile([128, 256], BF16, tag="wp_sb")
            w2_sb = pers.tile([64, 16], BF16, tag="w2_sb")
            b1_sb = pers.tile([64, 1], F32, tag="b1_sb")
            b2_sb = pers.tile([16, 1], F32, tag="b2_sb")
            brep_sb = pers.tile([128, HD], F32, tag="brep_sb")
            acc_sb = pers.tile([128, HD], F32, tag="acc_sb")
            ones_sb = pers.tile([128, 1], BF16, tag="ones_sb")
            ident = pers.tile([128, 128], BF16, tag="ident")
            zer64 = pers.tile([128, 64], BF16, tag="zer64")
            zer124 = pers.tile([128, 124], BF16, tag="zer124")
            zerf60 = pers.tile([128, 60], F32, tag="zerf60")
            zini = pers.tile([128, 128], BF16, tag="zini")

            nc.sync.dma_start(out=wbig_sb[:, :], in_=w_big[:, :])
            nc.sync.dma_start(out=wp_sb[:, :], in_=wp[:, :])
            nc.sync.dma_start(out=w2_sb[:, :], in_=w2[:, :])
            nc.sync.dma_start(out=b1_sb[:, :], in_=b1[:, :])
            nc.sync.dma_start(out=b2_sb[:, :], in_=b2[:, :])
            nc.sync.dma_start(out=brep_sb[:, :], in_=bias_rep[:, :])
            nc.sync.dma_start(out=ident[:, :], in_=ident_p[:, :])
            nc.vector.memset(acc_sb[:, :], 0.0)
            nc.vector.memset(ones_sb[:, :], 1.0)
            nc.vector.memset(zer64[:, :], 0.0)
            nc.vector.memset(zer124[:, :], 0.0)
            nc.vector.memset(zerf60[:, :], 0.0)
            nc.vector.memset(zini[:, :], 0.0)
            nc.sync.dma_start(out=contrib[NW * WN:NW * WN + 128, :],
                              in_=zini[:, :])
            nc.gpsimd.load_library(library_config.mlp)

            # ---------------- stage 0: node tables ------------------------
            XC = 512
            for c0 in range(0, NPAD, XC):
                cw_ = min(XC, NPAD - c0)
                xc = io.tile([IN, XC], BF16, tag="xc")
                nc.sync.dma_start(out=xc[:, :cw_], in_=xT[:, c0:c0 + cw_])
                for cc in range(0, cw_, 128):
                    pm = psp.tile([128, HER], F32, tag="mm")
                    nc.tensor.matmul(pm[:, :], lhsT=xc[:, cc:cc + 128],
                                     rhs=wbig_sb[:, :], start=True, stop=True)
                    hrow = io.tile([128, HE], BF16, tag="hrow")
                    erow = io.tile([128, 4], F32, tag="erow")
                    nc.scalar.activation(hrow[:, :], pm[:, 0:HE], AF.Copy)
                    nc.vector.tensor_copy(erow[:, :], pm[:, HE:HER])
                    r0 = c0 + cc
                    nc.sync.dma_start(out=h_tab[r0:r0 + 128, 0:HE],
                                      in_=hrow[:, :])
                    nc.sync.dma_start(out=h_tab[r0:r0 + 128, HE:HROW],
                                      in_=zer124[:, :])
                    nc.sync.dma_start(out=er_tab[r0:r0 + 128, :],
                                      in_=erow[:, :])

            # er_loc: per-window [128,1] indirect fetch + stage
            for w in range(NW):
                iw = io.tile([128, 1], I32, tag="iw")
                nc.sync.dma_start(out=iw[:, :],
                                  in_=idxW[w * 128:(w + 1) * 128, :])
                ew = io.tile([128, 4], F32, tag="ew")
                nc.gpsimd.indirect_dma_start(
                    out=ew[:, :], out_offset=None, in_=er_tab[:, :],
                    in_offset=bass.IndirectOffsetOnAxis(ap=iw[:, :], axis=0))
                nc.sync.dma_start(out=er_loc[w * WN:(w + 1) * WN, 0:4],
                                  in_=ew[:, :])
                nc.sync.dma_start(out=er_loc[w * WN:(w + 1) * WN, 4:64],
                                  in_=zerf60[:, :])

            # ---------------- phase A -------------------------------------
            with tc.tile_pool(name="bigA", bufs=2) as bigA:
                for it in range(NIT):
                    w0 = it * PAIR
                    nwin = min(PAIR, NW - w0)
                    nL, nH = nwin * WE_L, nwin * WE_H
                    KS = nwin * NCH
                    il = io.tile([128, PAIR * WE_L // 16], I16, tag="il")
                    ih = io.tile([128, PAIR * WE_H // 16], I16, tag="ih")
                    ie = io.tile([128, PAIR * NCH * 8], I16, tag="ie")
                    nc.sync.dma_start(out=il[:, :], in_=idxL_v[:, it, :])
                    nc.sync.dma_start(out=ih[:, :], in_=idxH_v[:, it, :])
                    nc.sync.dma_start(out=ie[:, :], in_=idxE_v[:, it, :])
                    bsb = bigA.tile([128, PAIR, NCH * 128], BF16, tag="bsb")
                    nc.sync.dma_start(out=bsb[:, :nwin, :],
                                      in_=Bc_v[:, w0:w0 + nwin, :])

                    G = bigA.tile([128, PAIR * NCH, HROW], BF16, tag="G")
                    Er = bigA.tile([128, PAIR * NCH, 64], F32, tag="Er")
                    lo_end_h = min(THR, NPAD)
                    hi_start_h = min(THR, NPAD - 128)
                    nc.gpsimd.dma_gather(
                        out_ap=G[:, 0:nL // 128, :], in_ap=h_tab[0:lo_end_h, :],
                        idxs_ap=il[:, 0:nL // 16], num_idxs=nL,
                        num_idxs_reg=nL, elem_size=HROW,
                        transpose=False, single_packet=False)
                    nc.gpsimd.dma_gather(
                        out_ap=G[:, nL // 128:KS, :], in_ap=h_tab[hi_start_h:, :],
                        idxs_ap=ih[:, 0:nH // 16], num_idxs=nH,
                        num_idxs_reg=nH, elem_size=HROW,
                        transpose=False, single_packet=False)
                    nc.gpsimd.dma_gather(
                        out_ap=Er[:, 0:KS, :], in_ap=er_loc[:, :],
                        idxs_ap=ie[:, 0:KS * 8], num_idxs=KS * 128,
                        num_idxs_reg=KS * 128, elem_size=64,
                        transpose=False, single_packet=False)

                    ee = io.tile([128, PAIR * NCH, 4], F32, tag="ee")
                    lr = io.tile([128, PAIR * NCH, 4], F32, tag="lr")
                    s_bf = io.tile([128, PAIR * NCH, 4], BF16, tag="s_bf")
                    nc.vector.tensor_copy(ee[:, :KS, :], G[:, :KS, HD:HE])
                    nc.vector.tensor_tensor(out=ee[:, :KS, :],
                                            in0=ee[:, :KS, :],
                                            in1=Er[:, :KS, 0:4], op=ALU.add)
                    nc.vector.tensor_scalar(lr[:, :KS, :], ee[:, :KS, :],
                                            NEG_SLOPE, None, ALU.mult)
                    nc.vector.tensor_tensor(out=ee[:, :KS, :],
                                            in0=ee[:, :KS, :],
                                            in1=lr[:, :KS, :], op=ALU.max)
                    nc.scalar.activation(s_bf[:, :KS, :], ee[:, :KS, :],
                                         AF.Exp)

                    M = bigA.tile([128, PAIR * NCH, HE], BF16, tag="M")
                    sb_ap = s_bf[:, :, :]
                    s_b = bass.AP(tensor=sb_ap.tensor, offset=sb_ap.offset,
                                  ap=[sb_ap.ap[0], [4, KS], [1, 4], [0, 64]])
                    nc.vector.tensor_tensor(
                        out=M[:, :KS, 0:HD].rearrange("p k (h d) -> p k h d",
                                                      h=4),
                        in0=G[:, :KS, 0:HD].rearrange("p k (h d) -> p k h d",
                                                      h=4),
                        in1=s_b, op=ALU.mult)
                    nc.vector.tensor_copy(M[:, :KS, HD:HE], s_bf[:, :KS, :])

                    for wi in range(nwin):
                        w = w0 + wi
                        pm = psp.tile([128, HER], F32, tag="mm")
                        for j in range(NCH):
                            gs = (wi * NCHL + j if j < NCHL
                                  else nwin * NCHL + wi * NCHH + (j - NCHL))
                            nc.tensor.matmul(
                                pm[:, 0:HE],
                                lhsT=bsb[:, wi, j * 128:(j + 1) * 128],
                                rhs=M[:, gs, :],
                                start=(j == 0), stop=(j == NCH - 1))
                        rz = io.tile([128, 4], F32, tag="rz")
                        nc.vector.tensor_scalar(rz[:, :], pm[:, HD:HE], 1e-20,
                                                None, ALU.add)
                        nc.vector.reciprocal(rz[:, :], rz[:, :])
                        hp = io.tile([128, HD], F32, tag="hp")
                        rz_ap = rz[:, :]
                        rz_b = bass.AP(tensor=rz_ap.tensor, offset=rz_ap.offset,
                                       ap=[rz_ap.ap[0], [1, 4], [0, 64]])
                        nc.vector.tensor_tensor(
                            out=hp[:, :].rearrange("p (h d) -> p h d", h=4),
                            in0=pm[:, 0:HD].rearrange("p (h d) -> p h d", h=4),
                            in1=rz_b, op=ALU.mult)
                        nc.vector.tensor_tensor(out=hp[:, :], in0=hp[:, :],
                                                in1=brep_sb[:, :], op=ALU.add)
                        # elu(x) = relu(x) + exp(min(x,0)) - 1
                        mn = io.tile([128, HD], F32, tag="mn")
                        rl = io.tile([128, HD], F32, tag="rl")
                        nc.vector.tensor_scalar(mn[:, :], hp[:, :], 0.0, None,
                                                ALU.min)
                        nc.scalar.activation(mn[:, :], mn[:, :], AF.Exp)
                        nc.scalar.activation(rl[:, :], hp[:, :], AF.Relu)
                        nc.vector.tensor_tensor(out=hp[:, :], in0=mn[:, :],
                                                in1=rl[:, :], op=ALU.add)
                        nc.vector.tensor_scalar(hp[:, :], hp[:, :], -1.0, None,
                                                ALU.add)
                        nc.vector.tensor_tensor(out=acc_sb[:, :],
                                                in0=acc_sb[:, :],
                                                in1=hp[:, :], op=ALU.add)
                        hpb = io.tile([128, HD], BF16, tag="hpb")
                        nc.scalar.activation(hpb[:, :], hp[:, :], AF.Copy)
                        hT = io.tile([128, 2, 128], BF16, tag="hT")
                        for j in range(2):
                            pt = psp.tile([128, 128], BF16, tag="tp")
                            nc.tensor.transpose(
                                out=pt[:, :],
                                in_=hpb[:, j * 128:(j + 1) * 128],
                                identity=ident[:, :])
                            nc.scalar.activation(hT[:, j, :], pt[:, :], AF.Copy)
                        pp = psp.tile([128, 128], F32, tag="tp")
                        for j in range(2):
                            nc.tensor.matmul(pp[:, :], lhsT=hT[:, j, :],
                                             rhs=wp_sb[:, j * 128:(j + 1) * 128],
                                             start=(j == 0), stop=(j == 1))
                        pb = io.tile([128, 128], BF16, tag="pb")
                        nc.scalar.activation(pb[:, :], pp[:, :], AF.Copy)
                        nc.sync.dma_start(
                            out=contrib[w * WN:(w + 1) * WN, 0:64],
                            in_=pb[:, 0:64])
                        nc.sync.dma_start(
                            out=contrib[w * WN:(w + 1) * WN, 64:128],
                            in_=zer64[:, :])
                        nc.sync.dma_start(
                            out=pd_tab[w * WN:(w + 1) * WN, 0:64],
                            in_=pb[:, 64:128])
                        nc.sync.dma_start(
                            out=pd_tab[w * WN:(w + 1) * WN, 64:128],
                            in_=zer64[:, :])

            # critic partials -> contrib rows NW*WN .. +4
            s4b = io.tile([128, 2], BF16, tag="s4b")
            for j in range(2):
                accb = io.tile([128, 128], BF16, tag="accb")
                nc.scalar.activation(accb[:, :],
                                     acc_sb[:, j * 128:(j + 1) * 128], AF.Copy)
                pscol = psp.tile([128, 128], F32, tag="tp")
                nc.tensor.matmul(pscol[:, 0:1], lhsT=accb[:, :],
                                 rhs=ones_sb[:, :], start=True, stop=True)
                nc.vector.tensor_copy(s4b[:, j:j + 1], pscol[:, 0:1])
            base = NW * WN
            for ph in range(2):
                dst_ap = bass.AP(tensor=contrib.ap().tensor,
                                 offset=base * 128 + ph * 128,
                                 ap=[[1, 64], [256, 2]])
                nc.sync.dma_start(out=dst_ap, in_=s4b[ph * 64:(ph + 1) * 64, :])

            nc.gpsimd.collective_compute(
                "AllGather", ALU.bypass,
                replica_groups=[list(range(N_CORES))],
                ins=[contrib.ap().opt()], outs=[ps_tab.ap().opt()])

            # ---------------- phase B: edge MLP ---------------------------
            with tc.tile_pool(name="bigB", bufs=2) as bigB:
                for t in range(NB):
                    ips = io.tile([128, TB // 16], I16, tag="ips")
                    ipd = io.tile([128, TB // 16], I16, tag="ipd")
                    nc.sync.dma_start(out=ips[:, :],
                                      in_=idx_ps[t * 128:(t + 1) * 128, :])
                    nc.sync.dma_start(out=ipd[:, :],
                                      in_=idx_pd[t * 128:(t + 1) * 128, :])
                    ps_g = bigB.tile([128, 1, TB], BF16, tag="ps_g")
                    pd_g = bigB.tile([128, 1, TB], BF16, tag="pd_g")
                    rows_total = N_CORES * CROWS
                    lo_end = min(THR, rows_total)
                    hi_start = min(THR, rows_total - 128)
                    tab_in = (ps_tab[0:lo_end, :] if t < NBL
                              else ps_tab[hi_start:, :])
                    nc.gpsimd.dma_gather(out_ap=ps_g[:, :, :], in_ap=tab_in,
                                         idxs_ap=ips[:, :], num_idxs=TB,
                                         num_idxs_reg=TB, elem_size=128,
                                         transpose=True, single_packet=False)
                    nc.gpsimd.dma_gather(out_ap=pd_g[:, :, :],
                                         in_ap=pd_tab[:, :],
                                         idxs_ap=ipd[:, :], num_idxs=TB,
                                         num_idxs_reg=TB, elem_size=128,
                                         transpose=True, single_packet=False)
                    pre = bigB.tile([64, TB], BF16, tag="pre")
                    nc.vector.tensor_tensor(out=pre[:, :],
                                            in0=ps_g[0:64, 0, :],
                                            in1=pd_g[0:64, 0, :], op=ALU.add)
                    nc.scalar.activation(pre[:, :], pre[:, :], AF.Relu,
                                         bias=b1_sb[:, :])
                    lo = bigB.tile([16, TB], F32, tag="lo")
                    for q in range(TB // 512):
                        pl = psp.tile([16, 512], F32, tag="pl")
                        nc.tensor.matmul(pl[:, :], lhsT=w2_sb[:, :],
                                         rhs=pre[:, q * 512:(q + 1) * 512],
                                         start=True, stop=True)
                        if q % 2 == 0:
                            nc.scalar.activation(lo[:, q * 512:(q + 1) * 512],
                                                 pl[:, :], AF.Identity,
                                                 bias=b2_sb[:, :])
                        else:
                            nc.vector.tensor_scalar(
                                lo[:, q * 512:(q + 1) * 512], pl[:, :],
                                b2_sb[:, :], None, ALU.add)
                    nc.sync.dma_start(out=out_logits[:, t * TB:(t + 1) * TB],
                                      in_=lo[:, :])

            # ---------------- critic --------------------------------------
            sums = io.tile([32, 64], BF16, tag="sums")
            sums_src = bass.AP(tensor=ps_tab.ap().tensor,
                               offset=NW * WN * 128,
                               ap=[[CROWS * 128, N_CORES], [128, 4], [1, 64]])
            nc.sync.dma_start(out=sums[:, :], in_=sums_src)
            sel_sb = io.tile([32, 4], BF16, tag="sel_sb")
            nc.sync.dma_start(out=sel_sb[:, :], in_=selp[:, :])
            pg = psp.tile([128, 128], F32, tag="tp")
            nc.tensor.matmul(pg[0:64, 0:4], lhsT=sums[:, :], rhs=sel_sb[:, :],
                             start=True, stop=True)
            g_bf = io.tile([64, 4], BF16, tag="g_bf")
            nc.vector.tensor_copy(g_bf[:, :], pg[0:64, 0:4])
            cw1_sb = io.tile([64, 256], BF16, tag="cw1_sb")
            nc.sync.dma_start(out=cw1_sb[:, :], in_=cw1[:, :])
            ph_ = psp.tile([128, 128], F32, tag="tp")
            for r in range(4):
                nc.tensor.matmul(ph_[0:64, 0:1],
                                 lhsT=cw1_sb[:, r * 64:(r + 1) * 64],
                                 rhs=g_bf[:, r:r + 1], start=(r == 0),
                                 stop=(r == 3))
            cb1_t = io.tile([64, 1], F32, tag="cb1_t")
            nc.sync.dma_start(out=cb1_t[:, :], in_=cb1[:, :])
            hid = io.tile([64, 1], BF16, tag="hid")
            nc.scalar.activation(hid[:, :], ph_[0:64, 0:1], AF.Relu,
                                 bias=cb1_t[:, :])
            cw2_sb = io.tile([64, 1], BF16, tag="cw2_sb")
            nc.sync.dma_start(out=cw2_sb[:, :], in_=cw2[:, :])
            pv = psp.tile([128, 128], F32, tag="tp")
            nc.tensor.matmul(pv[0:1, 0:1], lhsT=hid[:, :], rhs=cw2_sb[:, :],
                             start=True, stop=True)
            cb2_t = io.tile([1, 1], F32, tag="cb2_t")
            nc.sync.dma_start(out=cb2_t[:, :], in_=cb2[:, :])
            sv = io.tile([1, 1], F32, tag="sv")
            nc.scalar.activation(sv[:, :], pv[0:1, 0:1], AF.Identity,
                                 bias=cb2_t[:, :])
            nc.sync.dma_start(out=out_sval[:, :], in_=sv[:, :])

    nc.compile()
    return nc


# ----------------------------------------------------------------------------
# entry point
# ----------------------------------------------------------------------------

def kernel(**inputs):
    from concourse.bass_utils import run_bass_kernel_spmd
    import kernel as _self
    inputs = {k: np.asarray(v) for k, v in inputs.items()}
    dims, per_core, slotB = _prep(**inputs)
    nc = build(dims)
    res = run_bass_kernel_spmd(nc, per_core, core_ids=list(range(N_CORES)))
    _self._last_result = res
    E = dims['E']
    logits = np.zeros((E, 16), np.float32)
    for k in range(N_CORES):
        lg = np.asarray(res.results[k]["out_logits"])
        valid = slotB[k] >= 0
        logits[slotB[k][valid]] = lg.T[valid]
    sval = np.float32(np.asarray(res.results[0]["out_sval"]).reshape(()))
    return logits, sval
